# revision 22
# baseline (speedup 1.0000x reference)
"""Trainium2 Bass kernel for nn_C4HierarchicalExecutor (scatter_memory).

Math: with SCALE=10 and NUM_BITS=20, attention scores between binary-encoded
addresses are max-separated by 2*SCALE^2/sqrt(20) ~= 44.7 per mismatched bit,
so softmax weights at any non-matching key are ~exp(-44.7) ~= 4e-20 -- far
below fp32 resolution against the weight 1.0 at the exact match.  Hence in
fp32 the reference reduces exactly to:
    reads[b]      = memory[read_addrs[b]]
    new_memory[a] = memory[a] for untouched a; for each written address
                    (sequentially) m <- m*(1-en) + v*en
i.e. a pure gather + single-cell scatter update ("scatter_memory").
(Verified bit-exact for reads and within 1 ulp for new_memory against the
jax reference; the 1 ulp comes from the reference's fused multiply-add.)

Device strategy (8 NeuronCores, memory axis sharded, 131072 cells/core):
  - the scalar HWDGE ring streams the core's 512KB memory shard DRAM->DRAM
    into the output (the memory-regime roofline traffic), while
  - the sync HWDGE ring loads the per-core compacted index and [alpha,beta]
    lists into SBUF, and
  - one gpsimd indirect DMA gathers the owned write cells (rows 0..k-1) and
    read cells (rows k..2k-1) from the input shard in a single shot,
  - a gpsimd tensor_scalar computes final = m*alpha + beta, and
  - a gpsimd indirect DMA scatters the finals into the output shard once the
    bulk copy has landed (write-after-write on the same cells).
  - reads exit via a sync-ring DMA of gather rows k..2k-1.
Rows a core does not own carry an out-of-range index that bounds_check
skips; padding rows replicate the first owned row (a same-value write
collision is harmless).  Host computes the tiny per-write alpha/beta
coefficients (composing duplicate addresses sequentially), slices shards,
and concatenates the 8 output shards.

k=16 slots cover any plausible random ownership (binomial(64, 1/8) per
core); if some core owns more rows the kernel transparently rebuilds with
k=64, which handles the fully-adversarial all-in-one-shard case.
"""

import os
from typing import Any

import numpy as np

N_CORES = 8
MEM_SIZE = 1 << 20
SHARD = MEM_SIZE // N_CORES  # 131072
SHARD_SHIFT = 17  # log2(SHARD)
B = 64  # batch of read/write addresses
K_COMPACT = 16  # per-core compacted row capacity (fallback: B)
OOB = np.int32(1 << 28)  # any index > SHARD-1 is skipped by bounds_check

_cache: dict[Any, Any] = {}


# The Bass-emitted BIR embeds the builder's source filename/lineno in
# tensor debug info; compiling the builder from a fixed synthetic filename
# keeps the BIR byte-identical regardless of where this file lives, so the
# neuron compile cache hits from any working directory.
_BUILD_SRC = """
def _build_bass(k: int):
    import concourse.bass as bass
    import concourse.mybir as mybir

    f32 = mybir.dt.float32
    i32 = mybir.dt.int32

    nrows = 2 * k  # rows 0..k-1: write cells; rows k..2k-1: read cells

    # The 4 const-AP tiles Bass pre-fills in __init__ are unused by this
    # kernel (only activation-bias lowering reads them), but their MEMSETs
    # are the first "useful" instructions in the profile and anchor the
    # measured exec window ~1.3us before our first real instruction.
    # Suppress them for a cleaner (and honestly-accounted) kernel.
    _orig_memset = bass.BassEitherVectorEngine.memset
    bass.BassEitherVectorEngine.memset = lambda self, ap, c: None
    try:
        nc = bass.Bass(
            enable_partition_id=False,
            monotonic_sem_count=0,
            # Keep source paths out of the BIR so the NEFF cache hits from
            # any working directory.
            disable_frame_to_traceback=True,
        )
    finally:
        bass.BassEitherVectorEngine.memset = _orig_memset
    mem_in = nc.declare_dram_parameter("mem_in", [SHARD], f32, isOutput=False)
    idx = nc.declare_dram_parameter("idx", [nrows], i32, isOutput=False)
    ab = nc.declare_dram_parameter("ab", [k, 2], f32, isOutput=False)
    mem_out = nc.declare_dram_parameter("mem_out", [SHARD], f32, isOutput=True)
    reads_out = nc.declare_dram_parameter("reads_out", [k, 1], f32, isOutput=True)

    with (
        nc.sbuf_tensor([nrows, 1], i32) as idx_t,
        nc.sbuf_tensor([nrows, 1], f32) as g2,
        nc.sbuf_tensor([nrows, 2], f32) as ab_t,
        nc.sbuf_tensor([nrows, 1], f32) as fin,
        nc.semaphore("s_idx") as s_idx,
        nc.semaphore("s_ab") as s_ab,
        nc.semaphore("s_g") as s_g,
        nc.semaphore("s_bulk") as s_bulk,
        nc.semaphore("s_fin") as s_fin,
        nc.semaphore("s_done") as s_done,
        nc.semaphore("s_rdone") as s_rdone,
        nc.Block() as block,
    ):

        @block.sync
        def _(s):
            s.dma_start(out=idx_t[:], in_=idx[:, None]).then_inc(s_idx, 16)
            s.dma_start(out=ab_t[0:k, :], in_=ab[:]).then_inc(s_ab, 16)
            s.wait_ge(s_g, 16)
            s.dma_start(out=reads_out[:], in_=g2[k : 2 * k, 0:1]).then_inc(s_rdone, 16)

        @block.scalar
        def _(sc):
            # Bulk shard copy: DRAM -> DRAM on the ACT HWDGE ring, isolated
            # from both the sync ring (small loads) and the SWDGE ring
            # (indirect gather/scatter).
            sc.dma_start(out=mem_out[:], in_=mem_in[:]).then_inc(s_bulk, 16)

        @block.gpsimd
        def _(g):
            # One fused gather, one offset per partition: rows 0..k-1 are
            # the written cells, rows k..2k-1 the read cells.  Keeping the
            # write rows at base partition 0 keeps the compute + scatter APs
            # on partition 0, which real HW requires (DMA-only access like
            # reads_out tolerates the base-k start).  Rows carry either
            # an owned local index, a copy of the first owned row (padding),
            # or an out-of-range index that bounds_check skips.
            g.wait_ge(s_idx, 16)
            g.indirect_dma_start(
                out=g2[:],
                out_offset=None,
                in_=mem_in[:, None],
                in_offset=bass.IndirectOffsetOnAxis(ap=idx_t[:, 0:1], axis=0),
                bounds_check=SHARD - 1,
                oob_is_err=False,
            ).then_inc(s_g, 16)
            # final = (m * alpha) + beta in one op, computed here on gpsimd
            # to avoid the cross-engine wake + semaphore hops -- exactly the
            # reference's fp32 update arithmetic.
            g.wait_ge(s_g, 16)
            g.wait_ge(s_ab, 16)
            g.tensor_scalar(
                out=fin[0:k, 0:1],
                in0=g2[0:k, 0:1],
                scalar1=ab_t[0:k, 0:1],
                scalar2=ab_t[0:k, 1:2],
                op0=mybir.AluOpType.mult,
                op1=mybir.AluOpType.add,
            ).then_inc(s_fin, 1)
            # Scatter the final values once the bulk copy AND fin are done.
            # Padding rows rewrite the first owned cell with its own value.
            g.wait_ge(s_fin, 1)
            g.wait_ge(s_bulk, 16)
            g.indirect_dma_start(
                out=mem_out[:, None],
                out_offset=bass.IndirectOffsetOnAxis(ap=idx_t[0:k, 0:1], axis=0),
                in_=fin[0:k, 0:1],
                in_offset=None,
                bounds_check=SHARD - 1,
                oob_is_err=False,
            ).then_inc(s_done, 16)
            # No explicit s_done wait: the kernel epilogue's DMA drain waits
            # for outstanding transfers before semaphores are reset.


    return nc
"""

_ns: dict[str, Any] = {"SHARD": SHARD, "Any": Any}
exec(compile(_BUILD_SRC, "<nn_c4_scatter_memory_kernel>", "exec"), _ns)
_build_bass = _ns["_build_bass"]


def _get_nc(k: int):
    if ("nc", k) not in _cache:
        _cache[("nc", k)] = _build_bass(k)
    return _cache[("nc", k)]


def _compact(owner: np.ndarray, loc: np.ndarray, core: int, k: int):
    """Rows this core owns, compacted into k slots.

    Returns (idx_col[k] int32, rows[list of global b per slot]).  Padding
    replicates slot 0's index (harmless duplicate); all-OOB if none owned.
    """
    rows = np.nonzero(owner == core)[0]
    col = np.full(k, OOB, np.int32)
    n = len(rows)
    col[:n] = loc[rows]
    if n:
        col[n:] = col[0]
    return col, rows


def kernel(memory, read_addrs, write_addrs, write_values, write_enable):
    from concourse.bass_utils import run_bass_kernel_spmd

    memory = np.ascontiguousarray(np.asarray(memory, dtype=np.float32))
    ra = np.asarray(read_addrs).astype(np.int64)
    wa = np.asarray(write_addrs).astype(np.int64)
    wv = np.asarray(write_values, dtype=np.float32)
    we = np.asarray(write_enable, dtype=np.float32)

    # Per-write affine update coefficients: final = m*alpha + beta.
    # softmax at the written cell is exactly 1.0 in fp32, so w == enable and
    # one step is m*(1-en) + v*en.  Duplicate addresses compose sequentially;
    # only the last write of each address carries the (composed) update.
    one = np.float32(1.0)
    alpha_step = one - we  # fp32, exactly the reference's (1-w)
    beta_step = wv * we  # fp32, exactly the reference's v*w
    alpha = np.ones(B, np.float32)
    beta = np.zeros(B, np.float32)
    active = np.zeros(B, bool)
    groups: dict[int, list[int]] = {}
    for b in range(B):
        groups.setdefault(int(wa[b]), []).append(b)
    for a, bs in groups.items():
        if len(bs) == 1:
            bl = bs[0]
            alpha[bl] = alpha_step[bl]
            beta[bl] = beta_step[bl]
        else:
            A, Bv = 1.0, 0.0  # float64 composition (ulp-level vs reference)
            for b in bs:
                A = A * float(alpha_step[b])
                Bv = Bv * float(alpha_step[b]) + float(beta_step[b])
            bl = bs[-1]
            alpha[bl] = np.float32(A)
            beta[bl] = np.float32(Bv)
        active[bl] = True

    owner_r = (ra >> SHARD_SHIFT).astype(np.int64)
    loc_r = (ra & (SHARD - 1)).astype(np.int32)
    owner_w = np.where(active, wa >> SHARD_SHIFT, -1).astype(np.int64)
    loc_w = (wa & (SHARD - 1)).astype(np.int32)

    max_owned = 0
    for c in range(N_CORES):
        max_owned = max(max_owned, int((owner_r == c).sum()), int((owner_w == c).sum()))
    k = K_COMPACT if max_owned <= K_COMPACT else B

    in_maps = []
    r_rows_by_core = []
    for c in range(N_CORES):
        rd_col, r_rows = _compact(owner_r, loc_r, c, k)
        wr_col, w_rows = _compact(owner_w, loc_w, c, k)
        r_rows_by_core.append(r_rows)
        ab_c = np.zeros((k, 2), np.float32)
        n = len(w_rows)
        ab_c[:n, 0] = alpha[w_rows]
        ab_c[:n, 1] = beta[w_rows]
        if n:  # padding rows recompute slot 0's value
            ab_c[n:] = ab_c[0]
        in_maps.append(
            {
                "mem_in": np.ascontiguousarray(memory[c * SHARD : (c + 1) * SHARD]),
                "idx": np.ascontiguousarray(np.concatenate([wr_col, rd_col])),
                "ab": ab_c,
            }
        )

    nc = _get_nc(k)
    want_trace = bool(int(os.environ.get("KERNEL_TRACE", "0")))
    if want_trace:
        os.environ.pop("BASS_NEVER_TRACE", None)
    else:
        # A stray BASS_TRACE=1 in the environment would route through an
        # NTFF profiling hook this image does not ship; pin tracing off.
        os.environ["BASS_NEVER_TRACE"] = "1"
    res = run_bass_kernel_spmd(
        nc,
        in_maps,
        core_ids=list(range(N_CORES)),
        trace=want_trace,
    )
    _cache["last_results"] = res

    reads = np.zeros(B, np.float32)
    for c in range(N_CORES):
        rows = r_rows_by_core[c]
        if len(rows):
            reads[rows] = res.results[c]["reads_out"][: len(rows), 0]
    new_memory = np.concatenate([res.results[c]["mem_out"] for c in range(N_CORES)])
    return reads, new_memory


# revision 23
# speedup vs baseline: 1.0140x; 1.0140x over previous
"""Trainium2 Bass kernel for nn_C4HierarchicalExecutor (scatter_memory).

Math: with SCALE=10 and NUM_BITS=20, attention scores between binary-encoded
addresses are max-separated by 2*SCALE^2/sqrt(20) ~= 44.7 per mismatched bit,
so softmax weights at any non-matching key are ~exp(-44.7) ~= 4e-20 -- far
below fp32 resolution against the weight 1.0 at the exact match.  Hence in
fp32 the reference reduces exactly to:
    reads[b]      = memory[read_addrs[b]]
    new_memory[a] = memory[a] for untouched a; for each written address
                    (sequentially) m <- m*(1-en) + v*en
i.e. a pure gather + single-cell scatter update ("scatter_memory").
(Verified bit-exact for reads and within 1 ulp for new_memory against the
jax reference; the 1 ulp comes from the reference's fused multiply-add.)

Device strategy (8 NeuronCores, memory axis sharded, 131072 cells/core):
  - the scalar HWDGE ring streams the core's 512KB memory shard DRAM->DRAM
    into the output (the memory-regime roofline traffic), while
  - the sync HWDGE ring loads the per-core compacted index and [alpha,beta]
    lists into SBUF, and
  - one gpsimd indirect DMA gathers the owned write cells (rows 0..k-1) and
    read cells (rows k..2k-1) from the input shard in a single shot,
  - a gpsimd tensor_scalar computes final = m*alpha + beta, and
  - a gpsimd indirect DMA scatters the finals into the output shard once the
    bulk copy has landed (write-after-write on the same cells).
  - reads exit via a sync-ring DMA of gather rows k..2k-1.
Rows a core does not own carry an out-of-range index that bounds_check
skips; padding rows replicate the first owned row (a same-value write
collision is harmless).  Host computes the tiny per-write alpha/beta
coefficients (composing duplicate addresses sequentially), slices shards,
and concatenates the 8 output shards.

k=16 slots cover any plausible random ownership (binomial(64, 1/8) per
core); if some core owns more rows the kernel transparently rebuilds with
k=64, which handles the fully-adversarial all-in-one-shard case.
"""

import os
from typing import Any

import numpy as np

N_CORES = 8
MEM_SIZE = 1 << 20
SHARD = MEM_SIZE // N_CORES  # 131072
SHARD_SHIFT = 17  # log2(SHARD)
B = 64  # batch of read/write addresses
K_TIERS = (12, 16)  # per-core compacted row capacities (fallback: B)
OOB = np.int32(1 << 28)  # any index > SHARD-1 is skipped by bounds_check

_cache: dict[Any, Any] = {}


# The Bass-emitted BIR embeds the builder's source filename/lineno in
# tensor debug info; compiling the builder from a fixed synthetic filename
# keeps the BIR byte-identical regardless of where this file lives, so the
# neuron compile cache hits from any working directory.
_BUILD_SRC = """
def _build_bass(k: int):
    import concourse.bass as bass
    import concourse.mybir as mybir

    f32 = mybir.dt.float32
    i32 = mybir.dt.int32

    nrows = 2 * k  # rows 0..k-1: write cells; rows k..2k-1: read cells

    # The 4 const-AP tiles Bass pre-fills in __init__ are unused by this
    # kernel (only activation-bias lowering reads them), but their MEMSETs
    # are the first "useful" instructions in the profile and anchor the
    # measured exec window ~1.3us before our first real instruction.
    # Suppress them for a cleaner (and honestly-accounted) kernel.
    _orig_memset = bass.BassEitherVectorEngine.memset
    bass.BassEitherVectorEngine.memset = lambda self, ap, c: None
    try:
        nc = bass.Bass(
            enable_partition_id=False,
            monotonic_sem_count=0,
            # Keep source paths out of the BIR so the NEFF cache hits from
            # any working directory.
            disable_frame_to_traceback=True,
        )
    finally:
        bass.BassEitherVectorEngine.memset = _orig_memset
    mem_in = nc.declare_dram_parameter("mem_in", [SHARD], f32, isOutput=False)
    idx = nc.declare_dram_parameter("idx", [nrows], i32, isOutput=False)
    ab = nc.declare_dram_parameter("ab", [k, 2], f32, isOutput=False)
    mem_out = nc.declare_dram_parameter("mem_out", [SHARD], f32, isOutput=True)
    reads_out = nc.declare_dram_parameter("reads_out", [k, 1], f32, isOutput=True)

    with (
        nc.sbuf_tensor([nrows, 1], i32) as idx_t,
        nc.sbuf_tensor([nrows, 1], f32) as g2,
        nc.sbuf_tensor([nrows, 2], f32) as ab_t,
        nc.sbuf_tensor([nrows, 1], f32) as fin,
        nc.semaphore("s_idx") as s_idx,
        nc.semaphore("s_ab") as s_ab,
        nc.semaphore("s_g") as s_g,
        nc.semaphore("s_bulk") as s_bulk,
        nc.semaphore("s_fin") as s_fin,
        nc.semaphore("s_done") as s_done,
        nc.semaphore("s_rdone") as s_rdone,
        nc.Block() as block,
    ):

        @block.sync
        def _(s):
            s.dma_start(out=idx_t[:], in_=idx[:, None]).then_inc(s_idx, 16)
            s.dma_start(out=ab_t[0:k, :], in_=ab[:]).then_inc(s_ab, 16)
            s.wait_ge(s_g, 16)
            s.dma_start(out=reads_out[:], in_=g2[k : 2 * k, 0:1]).then_inc(s_rdone, 16)

        @block.scalar
        def _(sc):
            # Bulk shard copy: DRAM -> DRAM on the ACT HWDGE ring, isolated
            # from both the sync ring (small loads) and the SWDGE ring
            # (indirect gather/scatter).
            sc.dma_start(out=mem_out[:], in_=mem_in[:]).then_inc(s_bulk, 16)

        @block.gpsimd
        def _(g):
            # One fused gather, one offset per partition: rows 0..k-1 are
            # the written cells, rows k..2k-1 the read cells.  Keeping the
            # write rows at base partition 0 keeps the compute + scatter APs
            # on partition 0, which real HW requires (DMA-only access like
            # reads_out tolerates the base-k start).  Rows carry either
            # an owned local index, a copy of the first owned row (padding),
            # or an out-of-range index that bounds_check skips.
            g.wait_ge(s_idx, 16)
            g.indirect_dma_start(
                out=g2[:],
                out_offset=None,
                in_=mem_in[:, None],
                in_offset=bass.IndirectOffsetOnAxis(ap=idx_t[:, 0:1], axis=0),
                bounds_check=SHARD - 1,
                oob_is_err=False,
            ).then_inc(s_g, 16)
            # final = (m * alpha) + beta in one op, computed here on gpsimd
            # to avoid the cross-engine wake + semaphore hops -- exactly the
            # reference's fp32 update arithmetic.
            g.wait_ge(s_g, 16)
            g.wait_ge(s_ab, 16)
            g.tensor_scalar(
                out=fin[0:k, 0:1],
                in0=g2[0:k, 0:1],
                scalar1=ab_t[0:k, 0:1],
                scalar2=ab_t[0:k, 1:2],
                op0=mybir.AluOpType.mult,
                op1=mybir.AluOpType.add,
            ).then_inc(s_fin, 1)
            # Scatter the final values once the bulk copy AND fin are done.
            # Padding rows rewrite the first owned cell with its own value.
            g.wait_ge(s_fin, 1)
            g.wait_ge(s_bulk, 16)
            g.indirect_dma_start(
                out=mem_out[:, None],
                out_offset=bass.IndirectOffsetOnAxis(ap=idx_t[0:k, 0:1], axis=0),
                in_=fin[0:k, 0:1],
                in_offset=None,
                bounds_check=SHARD - 1,
                oob_is_err=False,
            ).then_inc(s_done, 16)
            # No explicit s_done wait: the kernel epilogue's DMA drain waits
            # for outstanding transfers before semaphores are reset.


    return nc
"""

_ns: dict[str, Any] = {"SHARD": SHARD, "Any": Any}
exec(compile(_BUILD_SRC, "<nn_c4_scatter_memory_kernel>", "exec"), _ns)
_build_bass = _ns["_build_bass"]


def _get_nc(k: int):
    if ("nc", k) not in _cache:
        _cache[("nc", k)] = _build_bass(k)
    return _cache[("nc", k)]


def _compact(owner: np.ndarray, loc: np.ndarray, core: int, k: int):
    """Rows this core owns, compacted into k slots.

    Returns (idx_col[k] int32, rows[list of global b per slot]).  Padding
    replicates slot 0's index (harmless duplicate); all-OOB if none owned.
    """
    rows = np.nonzero(owner == core)[0]
    col = np.full(k, OOB, np.int32)
    n = len(rows)
    col[:n] = loc[rows]
    if n:
        col[n:] = col[0]
    return col, rows


def kernel(memory, read_addrs, write_addrs, write_values, write_enable):
    from concourse.bass_utils import run_bass_kernel_spmd

    memory = np.ascontiguousarray(np.asarray(memory, dtype=np.float32))
    ra = np.asarray(read_addrs).astype(np.int64)
    wa = np.asarray(write_addrs).astype(np.int64)
    wv = np.asarray(write_values, dtype=np.float32)
    we = np.asarray(write_enable, dtype=np.float32)

    # Per-write affine update coefficients: final = m*alpha + beta.
    # softmax at the written cell is exactly 1.0 in fp32, so w == enable and
    # one step is m*(1-en) + v*en.  Duplicate addresses compose sequentially;
    # only the last write of each address carries the (composed) update.
    one = np.float32(1.0)
    alpha_step = one - we  # fp32, exactly the reference's (1-w)
    beta_step = wv * we  # fp32, exactly the reference's v*w
    alpha = np.ones(B, np.float32)
    beta = np.zeros(B, np.float32)
    active = np.zeros(B, bool)
    groups: dict[int, list[int]] = {}
    for b in range(B):
        groups.setdefault(int(wa[b]), []).append(b)
    for a, bs in groups.items():
        if len(bs) == 1:
            bl = bs[0]
            alpha[bl] = alpha_step[bl]
            beta[bl] = beta_step[bl]
        else:
            A, Bv = 1.0, 0.0  # float64 composition (ulp-level vs reference)
            for b in bs:
                A = A * float(alpha_step[b])
                Bv = Bv * float(alpha_step[b]) + float(beta_step[b])
            bl = bs[-1]
            alpha[bl] = np.float32(A)
            beta[bl] = np.float32(Bv)
        active[bl] = True

    owner_r = (ra >> SHARD_SHIFT).astype(np.int64)
    loc_r = (ra & (SHARD - 1)).astype(np.int32)
    owner_w = np.where(active, wa >> SHARD_SHIFT, -1).astype(np.int64)
    loc_w = (wa & (SHARD - 1)).astype(np.int32)

    max_owned = 0
    for c in range(N_CORES):
        max_owned = max(max_owned, int((owner_r == c).sum()), int((owner_w == c).sum()))
    k = B
    for tier in K_TIERS:
        if max_owned <= tier:
            k = tier
            break

    in_maps = []
    r_rows_by_core = []
    for c in range(N_CORES):
        rd_col, r_rows = _compact(owner_r, loc_r, c, k)
        wr_col, w_rows = _compact(owner_w, loc_w, c, k)
        r_rows_by_core.append(r_rows)
        ab_c = np.zeros((k, 2), np.float32)
        n = len(w_rows)
        ab_c[:n, 0] = alpha[w_rows]
        ab_c[:n, 1] = beta[w_rows]
        if n:  # padding rows recompute slot 0's value
            ab_c[n:] = ab_c[0]
        in_maps.append(
            {
                "mem_in": np.ascontiguousarray(memory[c * SHARD : (c + 1) * SHARD]),
                "idx": np.ascontiguousarray(np.concatenate([wr_col, rd_col])),
                "ab": ab_c,
            }
        )

    nc = _get_nc(k)
    want_trace = bool(int(os.environ.get("KERNEL_TRACE", "0")))
    if want_trace:
        os.environ.pop("BASS_NEVER_TRACE", None)
    else:
        # A stray BASS_TRACE=1 in the environment would route through an
        # NTFF profiling hook this image does not ship; pin tracing off.
        os.environ["BASS_NEVER_TRACE"] = "1"
    res = run_bass_kernel_spmd(
        nc,
        in_maps,
        core_ids=list(range(N_CORES)),
        trace=want_trace,
    )
    _cache["last_results"] = res

    reads = np.zeros(B, np.float32)
    for c in range(N_CORES):
        rows = r_rows_by_core[c]
        if len(rows):
            reads[rows] = res.results[c]["reads_out"][: len(rows), 0]
    new_memory = np.concatenate([res.results[c]["mem_out"] for c in range(N_CORES)])
    return reads, new_memory
